# revision 43
# baseline (speedup 1.0000x reference)
"""Causal multi-head self-attention with RoPE on 8 TRN2 NeuronCores.

Sharding: batch (2) x head-groups (4 groups of 4 heads) -> 8 cores.
Each core computes q/k/v projections for its 4 heads from its batch slice,
runs causal attention, and a partial o_proj against the matching Wo column
block; the host sums the 4 partials per batch (the o_proj all-reduce).

Device-side structure (v2 — chunk-pipelined):
  * All activations live transposed (feature-major): xT [1024,2048],
    QT/KT [256,2048]; every matmul contraction sits on the partition axis,
    no on-device transposes anywhere.
  * Main loop is software-pipelined at CHUNK level: iteration c emits
    proj(c) -> rope(c) -> attention(c-1). The PE flows from proj(c)
    straight into attention(c-1) while rope(c) runs on DVE/DMA — the PE
    never waits on RoPE. o_proj(j-1) matmuls ride inside attention(j)'s
    stream; o_proj(NJ-1) is the tail.
  * Scores are computed directly in transposed layout ST[sk,sq] = K @ Q^T.
    The two heads of a pair write the halves of ONE fused [128,1024] PSUM
    tile (2 banks) so off-diagonal tiles take a single 1024-col exp on the
    ACT engine (halves ACT instruction overhead — ACT paces the attention
    inner loop when the PE is at full clock).
  * V is stored [seq, 128] per head-slot as [ones|dims]: the PV output
    carries the softmax denominator on partitions 0:63 (where the DVE
    approx-reciprocal is legal — custom DVE ops NaN on base-64 inputs)
    and head dims on 64:127 (tensor_mul with PSUM in0 at base 64 + SBUF
    rcp at base 0 is HW-verified). The ones columns are constant and
    written once in the prologue. Softmax skips max-subtraction (scores
    bounded ~|6|; fp32 exp safe).
  * Causal masking: off-diagonal tiles cost nothing; diagonal tiles use
    narrowed matmul/exp column ranges plus an in-place affine_select on
    the GPSIMD engine (DVE untouched; GPSIMD cannot read PSUM).
  * DMA discipline (all learned from traces): ring entries block their
    issuing ENGINE when the ring fills, so the scalar/ACT ring carries
    only 2 early entries; one big DMA = one slow (~25-45GB/s) channel, so
    the critical first loads (wq + xt chunk 0) are k-pair-split across
    ~9 channels; per-chunk xt tiles + per-32-block rope-swap DMAs keep
    the byte-interval dep tracker precise (fused strided DMAs false-
    serialize against later proj writes). Late inputs (wo, xt c2/c3)
    are issued after the chunk-0/1 swaps. Bootstrap-chunk PSUM drains
    run on ACT (idle until the first exp); later chunks use DVE.
  * o_proj rides in 4-matmul bursts at i=1,3,5,7 of the next chunk's
    hp0 loop so the ACT exp-lookahead never drains; the tail chunk's
    y writes are split 8 ways over all 3 rings. Partial y is fp16
    (host sums in fp32) — halves the writeback.
  * Matmuls run in fp16 (e5m10: products exact in the fp32 PSUM
    accumulation; ~2x the float32r rate). PE sustains 2.4GHz when the
    stream is dense; HW duty-cycle throttling (ham k=4/8) costs ~10%.
"""
import numpy as np

import concourse.bass as bass
import concourse.mybir as mybir
import concourse.tile as tile
from concourse import bacc
from concourse.bass_utils import run_bass_kernel_spmd

F32 = mybir.dt.float32
F32R = mybir.dt.float32r
F16 = mybir.dt.float16
AF = mybir.ActivationFunctionType
ALU = mybir.AluOpType

DT_MM = F16          # matmul operand dtype: F32R or F16

BATCH, SEQ, DM = 2, 2048, 1024
NHEAD, DK = 16, 64
NCORES = 8
GROUPS = 4           # head groups (cores per batch)
HPC = 4              # heads per core
DH = HPC * DK        # 256 head dims per core
NK = DM // 128       # 8 contraction tiles over d_model
NJ = SEQ // 512      # 4 sq chunks
ROPE_THETA = 10000.0
LOOKAHEAD = 5        # ST/exp iterations emitted ahead of PV

TRACE = False        # set True to capture an NTFF profile on the next run
LAST_RESULTS = None  # BassKernelResults of the most recent run (for tooling)

_NC = None


def _round_f32r(a):
    """Round fp32 to fp32r (11-bit mantissa), RNE."""
    u = np.ascontiguousarray(a, dtype=np.float32).view(np.uint32)
    r = (u.astype(np.uint64) + 0x7FF + ((u >> 12) & 1)) & 0xFFFFF000
    return r.astype(np.uint32).view(np.float32)


def _round_mm(a):
    if DT_MM == F32R:
        return _round_f32r(a)
    return np.ascontiguousarray(a, dtype=np.float16)


def _build():
    nc = bacc.Bacc("TRN2", target_bir_lowering=False, debug=False)

    # weights arrive partition-major [128, k, d] so a k-pair DMA piece has
    # 1KB contiguous per-partition lines (full ~45GB/s per channel)
    xt_d = nc.dram_tensor("xt", [DM, SEQ], DT_MM, kind="ExternalInput").ap()
    wq_d = nc.dram_tensor("wq", [128, NK, DH], DT_MM, kind="ExternalInput").ap()
    wk_d = nc.dram_tensor("wk", [128, NK, DH], DT_MM, kind="ExternalInput").ap()
    wv_d = nc.dram_tensor("wv", [128, NK, DH], DT_MM, kind="ExternalInput").ap()
    wo_d = nc.dram_tensor("wo", [128, 2, DM], DT_MM, kind="ExternalInput").ap()
    cos_d = nc.dram_tensor("cosf", [128, SEQ], F16, kind="ExternalInput").ap()
    sin_d = nc.dram_tensor("sinf", [128, SEQ], F16, kind="ExternalInput").ap()
    y_d = nc.dram_tensor("y", [SEQ, DM], F16, kind="ExternalOutput").ap()

    # DVE-input view of a DT_MM AP (f32r bits are fp32 bits)
    VF = (lambda ap: ap.bitcast(F32)) if DT_MM == F32R else (lambda ap: ap)

    with tile.TileContext(nc) as tc:
        with tc.tile_pool(name="persist", bufs=1) as pp, \
             tc.tile_pool(name="ropep", bufs=3) as ropep, \
             tc.tile_pool(name="small", bufs=4) as sp, \
             tc.tile_pool(name="etp", bufs=LOOKAHEAD + 2) as etp, \
             tc.tile_pool(name="ysp", bufs=2) as ysp, \
             tc.tile_pool(name="ps_st", bufs=2, space="PSUM") as ps_st, \
             tc.tile_pool(name="ps_ot", bufs=2, space="PSUM") as ps_ot, \
             tc.tile_pool(name="ps_pj", bufs=2, space="PSUM") as ps_pj:

            # ---- resident tensors -------------------------------------
            # qt/kt live in ONE tile so the RoPE rotate-half swap covers
            # both with a single strided DMA per 32-partition block.
            qkt = pp.tile([128, 4 * SEQ], DT_MM, tag="qkt")
            qt = qkt[:, 0:2 * SEQ]
            kt = qkt[:, 2 * SEQ:4 * SEQ]
            v_sb = pp.tile([128, 16 * (HPC * 128)], DT_MM, tag="v")
            ht = pp.tile([128, 2 * SEQ], DT_MM, tag="ht")
            wo_sb = pp.tile([128, 2 * DM], DT_MM, tag="wo")
            # xt: one tile PER CHUNK so proj(c) depends only on its own DMA
            xt_c = [pp.tile([128, NK * 512], DT_MM, tag=f"xt{c}",
                            name=f"xt{c}")
                    for c in range(NJ)]
            wq_sb = pp.tile([128, NK * DH], DT_MM, tag="wq")
            wk_sb = pp.tile([128, NK * DH], DT_MM, tag="wk")
            wv_sb = pp.tile([128, NK * DH], DT_MM, tag="wv")
            cs_all = pp.tile([128, SEQ], F16, tag="cs")
            sn_all = pp.tile([128, SEQ], F16, tag="sn")
            tri = pp.tile([128, 128], DT_MM, tag="tri")

            xtv = xt_d.rearrange("(k p) s -> p k s", p=128)
            wq_s3 = wq_sb.rearrange("p (k d) -> p k d", d=DH)
            wk_s3 = wk_sb.rearrange("p (k d) -> p k d", d=DH)
            wv_s3 = wv_sb.rearrange("p (k d) -> p k d", d=DH)
            wo_s3 = wo_sb.rearrange("p (k d) -> p k d", d=DM)

            def xc3(c):
                return xt_c[c].rearrange("p (k s) -> p k s", s=512)

            # ---- input DMAs ------------------------------------------
            # All 5 engines issue at startup (each DIRECT2D costs ~700ns of
            # sequencer time; PE/ACT are idle until their first data lands
            # anyway). Pieces are ~128KB with >=1KB per-partition lines
            # (~45GB/s per channel), ordered by need-time: per-engine queue
            # slot s issues at ~7.2+0.7s us, lands ~2.8us later.
            def _wq(ks):
                return (wq_s3[:, ks, :], wq_d[:, ks, :])

            def _wk(ks):
                return (wk_s3[:, ks, :], wk_d[:, ks, :])

            def _wv(ks):
                return (wv_s3[:, ks, :], wv_d[:, ks, :])

            def _xt0(k, half=None):
                if half is None:
                    return (xc3(0)[:, k:k + 1, :], xtv[:, k:k + 1, 0:512])
                cs_ = slice(256 * half, 256 * half + 256)
                return (xc3(0)[:, k:k + 1, cs_], xtv[:, k:k + 1, cs_])

            def _xt1(k):
                return (xc3(1)[:, k:k + 1, :], xtv[:, k:k + 1, 512:1024])

            def _cs(c, t=None):
                sl = slice(512 * c, 512 * (c + (2 if t else 1)))
                return (cs_all[:, sl], cos_d[:, sl])

            def _sn(c, t=None):
                sl = slice(512 * c, 512 * (c + (2 if t else 1)))
                return (sn_all[:, sl], sin_d[:, sl])

            P2 = lambda a, b: slice(a, b)
            # Only SP(sync)/ACT(scalar) have HWDGE; gpsimd has SWDGE.
            # One engine's consecutive dma_starts land on different queues
            # and transfer CONCURRENTLY; the ~0.7us per-issue sequencer cost
            # is the startup constraint, so order by need-time. Scalar stops
            # after slot 7 (it must be free for the bootstrap proj drains).
            waves = {
                nc.sync: [_wq(P2(0, 1)), _wq(P2(1, 2)), _wq(P2(2, 4)),
                          _wq(P2(4, 6)), _wq(P2(6, 8)), _wk(P2(0, 2)),
                          _wk(P2(6, 8)), _sn(0), _wv(P2(6, 8)), _xt1(1),
                          _xt1(3), _xt1(5), _xt1(7), _sn(1), _sn(2, 't')],
                nc.scalar: [_xt0(0, 0), _xt0(2), _xt0(4), _xt0(6),
                            _wk(P2(2, 4)), _wv(P2(0, 2)), _cs(0)],
                nc.gpsimd: [_xt0(0, 1), _xt0(1), _xt0(3), _xt0(5),
                            _xt0(7), _wk(P2(4, 6)), _wv(P2(2, 4)),
                            _wv(P2(4, 6)), _xt1(0), _xt1(2), _xt1(4),
                            _xt1(6), _cs(1), _cs(2, 't')],
            }
            for si in range(15):
                for eng, lst in waves.items():
                    if si < len(lst):
                        o, i_ = lst[si]
                        eng.dma_start(out=o, in_=i_)

            h0, h1 = slice(0, 4), slice(4, 8)

            def late_loads(c):
                # late input loads (sync stays clear for rope swaps)
                if c == 0:
                    nc.gpsimd.dma_start(out=wo_s3[:, 0:1, :],
                                        in_=wo_d[:, 0:1, :])
                    nc.gpsimd.dma_start(out=wo_s3[:, 1:2, :],
                                        in_=wo_d[:, 1:2, :])
                    nc.gpsimd.dma_start(out=xc3(2)[:, h0, :],
                                        in_=xtv[:, h0, 1024:1536])
                    nc.gpsimd.dma_start(out=xc3(2)[:, h1, :],
                                        in_=xtv[:, h1, 1024:1536])
                elif c == 1:
                    nc.gpsimd.dma_start(out=xc3(3)[:, h0, :],
                                        in_=xtv[:, h0, 1536:2048])
                    nc.gpsimd.dma_start(out=xc3(3)[:, h1, :],
                                        in_=xtv[:, h1, 1536:2048])

            # ones-columns of v_sb are constant: write them ONCE here
            # (cols 0:64 of each of the 64 head-slots)
            v4 = v_sb.rearrange("p (s d) -> p s d", d=128)
            nc.gpsimd.memset(v4[:, :, 0:64], 1.0)
            # lower-triangular keep-mask for the causal diag blocks:
            # tri[p, c] = 1 where c >= p else 0 (multiplied into et on DVE)
            nc.gpsimd.memset(tri[:], 1.0)
            nc.gpsimd.affine_select(
                out=tri[:], in_=tri[:], compare_op=ALU.is_ge, fill=0.0,
                base=0, pattern=[[1, 128]], channel_multiplier=-1)

            def emit_proj_group(c, g, cp=None):
                """One projection group of chunk c.

                g 0..3: QK groups (chunk 0 orders them q,q,k,k so the wk
                DMAs get extra slack in the DMA-paced bootstrap); g 4..7:
                V seq-tile groups. cp: drain engine (ACT during bootstrap
                and when folded into an ACT-light attention stream, DVE
                otherwise).
                """
                xc = xt_c[c]
                if cp is None:
                    cp = nc.scalar if c <= 1 else nc.vector

                def drain(dst, src):
                    if cp is nc.scalar:
                        cp.copy(dst, src)
                    else:
                        cp.tensor_copy(dst, src)

                if g < 4:
                    if c == 0:
                        m, dst, w_sb = [(0, qt, wq_sb), (1, qt, wq_sb),
                                        (0, kt, wk_sb), (1, kt, wk_sb)][g]
                    else:
                        m, dst, w_sb = [(0, qt, wq_sb), (0, kt, wk_sb),
                                        (1, qt, wq_sb), (1, kt, wk_sb)][g]
                    ps = ps_pj.tile([128, 512], F32, tag="pj")
                    for k in range(NK):
                        nc.tensor.matmul(
                            ps[:],
                            w_sb[:, k * DH + m * 128: k * DH + (m + 1) * 128],
                            xc[:, k * 512:(k + 1) * 512],
                            start=(k == 0), stop=(k == NK - 1))
                    drain(dst[:, m * SEQ + c * 512: m * SEQ + (c + 1) * 512],
                          ps[:])
                else:
                    # V for seq tile 4c+t4, [sq 128, 4 head slots x 128].
                    # All slots hold [ones|dims]: PV outputs then carry the
                    # denominator on partitions 0:63 (where the DVE approx
                    # reciprocal is legal) and dims on 64:127. The ones
                    # cols were written once in the prologue.
                    t4 = g - 4
                    t = 4 * c + t4
                    ps = ps_pj.tile([128, 512], F32, tag="pj")
                    for k in range(NK):
                        nc.tensor.matmul(
                            ps[:, 0:DH],
                            xc[:, k * 512 + t4 * 128: k * 512 + t4 * 128 + 128],
                            wv_sb[:, k * DH:(k + 1) * DH],
                            start=(k == 0), stop=(k == NK - 1))
                    vt = v_sb[:, t * 512:(t + 1) * 512].rearrange(
                        "p (h d) -> p h d", d=128)
                    pv4 = ps[:, 0:DH].rearrange("p (h d) -> p h d", d=64)
                    drain(vt[:, :, 64:128], pv4[:])

            def emit_proj(c, cp=None):
                for g in range(8):
                    emit_proj_group(c, g, cp=cp)

            def emit_rope(c, ms=(0, 1)):
                # RoPE on QT/KT chunk c, in place. Head-dim pairs are
                # pre-permuted to [evens|odds] 32-row blocks (host-side
                # weight permute), so rotate-half = two 32-partition block
                # swaps done as 4 small SBUF DMAs per segment (per-block
                # DMAs keep the dep intervals precise — a fused strided DMA
                # false-serializes against later proj copies). Segment
                # order (q m0, k m0, q m1, k m1) lets attention's hp=0
                # start before the m=1 segments finish.
                cseg = slice(c * 512, (c + 1) * 512)
                segs = [s for s in ((0, qt), (2, kt), (1, qt), (3, kt))
                        if (s[0] % 2) in ms]
                for a, src in segs:
                    m = a % 2
                    base = m * SEQ + c * 512
                    seg = slice(base, base + 512)
                    t1 = ropep.tile([128, 512], F16, tag="t1")
                    nc.vector.tensor_mul(t1[:], VF(src[:, seg]),
                                         cs_all[:, cseg])
                    sw = ropep.tile([128, 512], DT_MM, tag="sw")
                    for blk in range(4):
                        sb_ = blk ^ 1
                        eng = nc.sync if blk % 2 == 0 else nc.gpsimd
                        eng.dma_start(
                            out=sw[blk * 32:(blk + 1) * 32, :],
                            in_=src[sb_ * 32:(sb_ + 1) * 32, seg])
                    sw2 = ropep.tile([128, 512], F16, tag="sw2")
                    nc.vector.tensor_mul(sw2[:], VF(sw[:]),
                                         sn_all[:, cseg])
                    nc.vector.tensor_add(src[:, seg], t1[:], sw2[:])

            def emit_oproj(j, last=False, t4s=range(4), act_drain=False):
                # Y[sq,dm] = H @ wo: lhsT = ht columns (weight reuse x2)
                for t4 in t4s:
                    ps0 = ps_pj.tile([128, 512], F32, tag="pj")
                    ps1 = ps_pj.tile([128, 512], F32, tag="pj")
                    for kk in range(2):
                        for n, psn in ((0, ps0), (1, ps1)):
                            nc.tensor.matmul(
                                psn[:],
                                ht[:, kk * SEQ + j * 512 + t4 * 128:
                                   kk * SEQ + j * 512 + (t4 + 1) * 128],
                                wo_sb[:, kk * DM + n * 512:
                                      kk * DM + (n + 1) * 512],
                                start=(kk == 0), stop=(kk == 1))
                    ys = ysp.tile([128, 1024], F16, tag="ys")
                    row = j * 512 + t4 * 128
                    if last:
                        # tail: drains split DVE+ACT (ACT idle after the
                        # last exp), y written by COLUMN halves so each
                        # DMA departs as soon as its own copy lands,
                        # spread over 5 rings (tensor takes the last piece
                        # once the PE is done with matmuls)
                        nc.vector.tensor_copy(ys[:, 0:512], ps0[:])
                        nc.scalar.copy(ys[:, 512:1024], ps1[:])
                        # gpsimd carries no DMAs near the end of the kernel:
                        # its expensive SWDGE drain then runs concurrently
                        # with the attention tail instead of after it
                        ring = (nc.sync, nc.sync, nc.sync, nc.sync,
                                nc.sync, nc.scalar, nc.sync, nc.scalar)
                        ring[2 * t4].dma_start(
                            out=y_d[row:row + 128, 0:512], in_=ys[:, 0:512])
                        ring[2 * t4 + 1].dma_start(
                            out=y_d[row:row + 128, 512:1024],
                            in_=ys[:, 512:1024])
                    else:
                        nc.vector.tensor_copy(ys[:, 0:512], ps0[:])
                        if act_drain:
                            nc.scalar.copy(ys[:, 512:1024], ps1[:])
                        else:
                            nc.vector.tensor_copy(ys[:, 512:1024], ps1[:])
                        eng = nc.gpsimd if j == 0 else nc.sync
                        eng.dma_start(out=y_d[row:row + 128, :], in_=ys[:])

            def emit_attn(js, fillers=None, mid=None, tail=False):
                # js: sq-chunk indices processed back-to-back per hp half
                # (merging small chunks keeps the ACT exp pipeline dense).
                # fillers: PE work (proj groups / o_proj bursts) consumed
                # one per odd tile index — the ACT exp lookahead hides the
                # matmul detours. mid: called between the hp halves (used
                # to slot the next chunk's rope segments into the DVE queue
                # where they can't head-of-line-block the masks). tail:
                # final chunk — fuse the hp1 normalize with o_proj per
                # 128-col block to shorten the serial tail.
                fillers = list(fillers or [])
                secs = [(hp, j) for hp in range(2) for j in js]
                tiles = [(s, i) for s in range(len(secs))
                         for i in range(4 * (secs[s][1] + 1))]
                ots = {}
                ets = {}

                def st_exp(t):
                    s, i = tiles[t]
                    hp, j = secs[s]
                    jb = hp * SEQ + j * 512
                    r = i - 4 * j          # >= 0 on diagonal tiles
                    c0 = 128 * r if r >= 0 else 0
                    ib = hp * SEQ + i * 128
                    st = ps_st.tile([128, 1024], F32, tag="st")
                    nc.tensor.matmul(st[:, c0:512],
                                     kt[0:64, ib:ib + 128],
                                     qt[0:64, jb + c0:jb + 512],
                                     start=True, stop=True)
                    nc.tensor.matmul(st[:, 512 + c0:1024],
                                     kt[64:128, ib:ib + 128],
                                     qt[64:128, jb + c0:jb + 512],
                                     start=True, stop=True)
                    et = etp.tile([128, 1024], DT_MM, tag="et")
                    if r < 0 or c0 == 0:
                        # one fused 1024-col exp
                        nc.scalar.activation(et[:], st[:],
                                             AF.Exp, scale=0.125)
                    else:
                        # diag: single exp over both narrowed halves
                        # via a 3D AP (one ACT instr, not two)
                        st3 = st.rearrange("p (b q) -> p b q", q=512)
                        et3 = et.rearrange("p (b q) -> p b q", q=512)
                        nc.scalar.activation(et3[:, :, c0:512],
                                             st3[:, :, c0:512],
                                             AF.Exp, scale=0.125)
                    if r >= 0:
                        # zero above-diagonal inside the [128,128] diag
                        # block: multiply by the tri keep-mask on DVE
                        # (fp16 2x mode, ~170ns; keeps GPSIMD clear)
                        for b0 in (c0, 512 + c0):
                            nc.vector.tensor_mul(
                                et[:, b0:b0 + 128],
                                et[:, b0:b0 + 128], tri[:])
                    ets[t] = (et, c0)

                def pv(t):
                    s, i = tiles[t]
                    hp, j = secs[s]
                    nlive = 4 * (j + 1)
                    if i == 0:
                        ots[s] = (ps_ot.tile([128, 512], F32, tag="ot",
                                             name=f"otA{s}"),
                                  ps_ot.tile([128, 512], F32, tag="ot",
                                             name=f"otB{s}"))
                    otA, otB = ots[s]
                    et, c0 = ets.pop(t)
                    vb = i * (HPC * 128) + 2 * hp * 128
                    nc.tensor.matmul(otA[:, c0:512],
                                     v_sb[:, vb:vb + 128],
                                     et[:, c0:512],
                                     start=(i == 0), stop=(i == nlive - 1))
                    nc.tensor.matmul(otB[:, c0:512],
                                     v_sb[:, vb + 128:vb + 256],
                                     et[:, 512 + c0:1024],
                                     start=(i == 0), stop=(i == nlive - 1))

                def finish(s):
                    # normalize: denom rides rows 0:63, dims rows 64:127.
                    # rcp is base-0-aligned; the mul reads dims at base 64
                    # (both constructs HW-verified).
                    hp, j = secs[s]
                    jb = hp * SEQ + j * 512
                    otA, otB = ots.pop(s)
                    if tail and s == len(secs) - 1:
                        # last chunk: normalize per 128-col block feeding
                        # o_proj t4 bursts; the DVE normalize runs one
                        # block AHEAD of the PE's o_proj so neither waits
                        def norm_t4(t4):
                            csl = slice(t4 * 128, (t4 + 1) * 128)
                            hsl = slice(jb + t4 * 128, jb + (t4 + 1) * 128)
                            for ot, rows in ((otA, slice(0, 64)),
                                             (otB, slice(64, 128))):
                                rcp = sp.tile([64, 128], F32, tag="rcpt")
                                nc.vector.reciprocal_approx_fast(
                                    rcp[:], ot[0:64, csl])
                                nc.vector.tensor_mul(ht[rows, hsl],
                                                     ot[64:128, csl], rcp[:])
                        norm_t4(0)
                        for t4 in range(4):
                            if t4 < 3:
                                norm_t4(t4 + 1)
                            emit_oproj(j, last=True, t4s=(t4,))
                    else:
                        for ot, rows in ((otA, slice(0, 64)),
                                         (otB, slice(64, 128))):
                            rcp = sp.tile([64, 512], F32, tag="rcp")
                            nc.vector.reciprocal_approx_fast(rcp[:],
                                                             ot[0:64, :])
                            nc.vector.tensor_mul(ht[rows, jb:jb + 512],
                                                 ot[64:128, :], rcp[:])

                # flat tile stream: the exp lookahead spans section (hp/j)
                # boundaries, so the ACT pipeline never refills from empty
                # at an hp transition
                ntiles = len(tiles)
                stride = 2
                if fillers:
                    stride = max(2, (ntiles // len(fillers)) & ~1)
                for t in range(min(LOOKAHEAD, ntiles)):
                    st_exp(t)
                for t in range(ntiles):
                    if t + LOOKAHEAD < ntiles:
                        st_exp(t + LOOKAHEAD)
                    pv(t)
                    s, i = tiles[t]
                    # filler PE work rides inside this stream in small
                    # bursts (spread over the whole phase) so the ACT exp
                    # lookahead never drains during the matmul detours
                    if fillers and t % stride == 1:
                        fillers.pop(0)()
                    if i == 4 * (secs[s][1] + 1) - 1:
                        finish(s)
                        if mid is not None and s == len(secs) // 2 - 1:
                            mid()

            # ---- chunk-pipelined main loop ----------------------------
            # attn(0) is tiny (all-diagonal, mask/exp-paced) — merged into
            # attn(1)'s stream. proj(3) + the o_proj bursts ride as fillers
            # inside the attention streams (the only standalone PE phases
            # left are proj(0..2)); rope segments are emitted at the hp
            # boundaries so their swap-muls never head-of-line-block the
            # next masks on the DVE queue.
            emit_proj(0)
            emit_rope(0)
            late_loads(0)
            emit_proj(1)
            emit_rope(1)
            late_loads(1)
            # proj(2) drains on ACT (idle until attn01's first exps): on
            # the DVE they head-of-line-block behind rope(1)'s swap-muls
            emit_proj(2, cp=nc.scalar)
            fill01 = [lambda g=g: emit_proj_group(3, g, cp=nc.scalar)
                      for g in range(8)]
            fill01 += [lambda t=t: emit_oproj(0, t4s=(t,), act_drain=True)
                       for t in range(4)]
            emit_attn([0, 1], fillers=fill01,
                      mid=lambda: emit_rope(2, ms=(0,)))
            emit_rope(2, ms=(1,))
            emit_attn([2],
                      fillers=[lambda t=t: emit_oproj(1, t4s=(t,))
                               for t in range(4)],
                      mid=lambda: emit_rope(3, ms=(0,)))
            emit_rope(3, ms=(1,))
            emit_attn([3],
                      fillers=[lambda t=t: emit_oproj(2, t4s=(t,))
                               for t in range(4)],
                      tail=True)

    nc.compile()
    return nc


def _prep_inputs(x, Wq, Wk, Wv, Wo, token_positions):
    x = np.asarray(x, dtype=np.float32)
    Wq = np.asarray(Wq, dtype=np.float32)
    Wk = np.asarray(Wk, dtype=np.float32)
    Wv = np.asarray(Wv, dtype=np.float32)
    Wo = np.asarray(Wo, dtype=np.float32)
    pos = np.asarray(token_positions).astype(np.float32)

    inv = 1.0 / (ROPE_THETA ** (np.arange(0, DK, 2, dtype=np.float32) / DK))
    freqs = pos[:, None] * inv[None, :]              # [SEQ, 32]
    cos_t, sin_t = np.cos(freqs).T, np.sin(freqs).T  # [32, SEQ]
    cosf = np.ascontiguousarray(np.tile(cos_t, (4, 1)), dtype=np.float16)
    sinf = np.tile(sin_t, (4, 1)).astype(np.float32)
    sinf[0:32] *= -1.0   # evens block gets -sin; odds +sin
    sinf[64:96] *= -1.0
    sinf = np.ascontiguousarray(sinf.astype(np.float16))

    def _pkd(wT):
        # [K*128, D] (contraction-major) -> [128, K, D] partition-major so
        # each k-pair DMA piece is >=1KB-contiguous per partition
        k = wT.shape[0] // 128
        return np.ascontiguousarray(
            _round_mm(wT).reshape(k, 128, -1).transpose(1, 0, 2))

    perm = np.concatenate([np.arange(0, 64, 2), np.arange(1, 64, 2)])
    in_maps = []
    for c in range(NCORES):
        b, g = divmod(c, GROUPS)
        rows = slice(g * DH, (g + 1) * DH)
        wq_s = Wq[rows, :].reshape(HPC, DK, DM)[:, perm, :].reshape(DH, DM)
        wk_s = Wk[rows, :].reshape(HPC, DK, DM)[:, perm, :].reshape(DH, DM)
        in_maps.append({
            "xt": _round_mm(x[b].T),
            "wq": _pkd(wq_s.T),
            "wk": _pkd(wk_s.T),
            "wv": _pkd(Wv[rows, :].T),
            "wo": _pkd(Wo[:, rows].T),
            "cosf": cosf,
            "sinf": sinf,
        })
    return in_maps


def kernel(x, Wq, Wk, Wv, Wo, token_positions):
    global _NC, LAST_RESULTS
    if _NC is None:
        _NC = _build()
    in_maps = _prep_inputs(x, Wq, Wk, Wv, Wo, token_positions)
    res = run_bass_kernel_spmd(_NC, in_maps, list(range(NCORES)), trace=TRACE)
    LAST_RESULTS = res
    y = np.empty((BATCH, SEQ, DM), dtype=np.float32)
    for b in range(BATCH):
        acc = res.results[4 * b]["y"].astype(np.float32)
        for g in range(1, GROUPS):
            acc += res.results[4 * b + g]["y"].astype(np.float32)
        y[b] = acc
    return y



# revision 46
# speedup vs baseline: 1.0267x; 1.0267x over previous
"""Causal multi-head self-attention with RoPE on 8 TRN2 NeuronCores.

Sharding: batch (2) x head-groups (4 groups of 4 heads) -> 8 cores.
Each core computes q/k/v projections for its 4 heads from its batch slice,
runs causal attention, and a partial o_proj against the matching Wo column
block; the host sums the 4 partials per batch (the o_proj all-reduce).

Device-side structure (v2 — chunk-pipelined):
  * All activations live transposed (feature-major): xT [1024,2048],
    QT/KT [256,2048]; every matmul contraction sits on the partition axis,
    no on-device transposes anywhere.
  * Main loop is software-pipelined at CHUNK level: iteration c emits
    proj(c) -> rope(c) -> attention(c-1). The PE flows from proj(c)
    straight into attention(c-1) while rope(c) runs on DVE/DMA — the PE
    never waits on RoPE. o_proj(j-1) matmuls ride inside attention(j)'s
    stream; o_proj(NJ-1) is the tail.
  * Scores are computed directly in transposed layout ST[sk,sq] = K @ Q^T.
    The two heads of a pair write the halves of ONE fused [128,1024] PSUM
    tile (2 banks) so off-diagonal tiles take a single 1024-col exp on the
    ACT engine (halves ACT instruction overhead — ACT paces the attention
    inner loop when the PE is at full clock).
  * V is stored [seq, 128] per head-slot as [ones|dims]: the PV output
    carries the softmax denominator on partitions 0:63 (where the DVE
    approx-reciprocal is legal — custom DVE ops NaN on base-64 inputs)
    and head dims on 64:127 (tensor_mul with PSUM in0 at base 64 + SBUF
    rcp at base 0 is HW-verified). The ones columns are constant and
    written once in the prologue. Softmax skips max-subtraction (scores
    bounded ~|6|; fp32 exp safe).
  * Causal masking: off-diagonal tiles cost nothing; diagonal tiles use
    narrowed matmul/exp column ranges plus an in-place affine_select on
    the GPSIMD engine (DVE untouched; GPSIMD cannot read PSUM).
  * DMA discipline (all learned from traces): ring entries block their
    issuing ENGINE when the ring fills, so the scalar/ACT ring carries
    only 2 early entries; one big DMA = one slow (~25-45GB/s) channel, so
    the critical first loads (wq + xt chunk 0) are k-pair-split across
    ~9 channels; per-chunk xt tiles + per-32-block rope-swap DMAs keep
    the byte-interval dep tracker precise (fused strided DMAs false-
    serialize against later proj writes). Late inputs (wo, xt c2/c3)
    are issued after the chunk-0/1 swaps. Bootstrap-chunk PSUM drains
    run on ACT (idle until the first exp); later chunks use DVE.
  * o_proj rides in 4-matmul bursts at i=1,3,5,7 of the next chunk's
    hp0 loop so the ACT exp-lookahead never drains; the tail chunk's
    y writes are split 8 ways over all 3 rings. Partial y is fp16
    (host sums in fp32) — halves the writeback.
  * Matmuls run in fp16 (e5m10: products exact in the fp32 PSUM
    accumulation; ~2x the float32r rate). PE sustains 2.4GHz when the
    stream is dense; HW duty-cycle throttling (ham k=4/8) costs ~10%.
"""
import numpy as np

import concourse.bass as bass
import concourse.mybir as mybir
import concourse.tile as tile
from concourse import bacc
from concourse.bass_utils import run_bass_kernel_spmd

F32 = mybir.dt.float32
F32R = mybir.dt.float32r
F16 = mybir.dt.float16
AF = mybir.ActivationFunctionType
ALU = mybir.AluOpType

DT_MM = F16          # matmul operand dtype: F32R or F16

BATCH, SEQ, DM = 2, 2048, 1024
NHEAD, DK = 16, 64
NCORES = 8
GROUPS = 4           # head groups (cores per batch)
HPC = 4              # heads per core
DH = HPC * DK        # 256 head dims per core
NK = DM // 128       # 8 contraction tiles over d_model
NJ = SEQ // 512      # 4 sq chunks
ROPE_THETA = 10000.0
LOOKAHEAD = 5        # ST/exp iterations emitted ahead of PV

TRACE = False        # set True to capture an NTFF profile on the next run
LAST_RESULTS = None  # BassKernelResults of the most recent run (for tooling)

_NC = None


def _round_f32r(a):
    """Round fp32 to fp32r (11-bit mantissa), RNE."""
    u = np.ascontiguousarray(a, dtype=np.float32).view(np.uint32)
    r = (u.astype(np.uint64) + 0x7FF + ((u >> 12) & 1)) & 0xFFFFF000
    return r.astype(np.uint32).view(np.float32)


def _round_mm(a):
    if DT_MM == F32R:
        return _round_f32r(a)
    return np.ascontiguousarray(a, dtype=np.float16)


def _build():
    nc = bacc.Bacc("TRN2", target_bir_lowering=False, debug=False)

    # weights arrive partition-major [128, k, d] so a k-pair DMA piece has
    # 1KB contiguous per-partition lines (full ~45GB/s per channel)
    xt_d = nc.dram_tensor("xt", [DM, SEQ], DT_MM, kind="ExternalInput").ap()
    wq_d = nc.dram_tensor("wq", [128, NK, DH], DT_MM, kind="ExternalInput").ap()
    wk_d = nc.dram_tensor("wk", [128, NK, DH], DT_MM, kind="ExternalInput").ap()
    wv_d = nc.dram_tensor("wv", [128, NK, DH], DT_MM, kind="ExternalInput").ap()
    wo_d = nc.dram_tensor("wo", [128, 2, DM], DT_MM, kind="ExternalInput").ap()
    cos_d = nc.dram_tensor("cosf", [128, SEQ], F16, kind="ExternalInput").ap()
    sin_d = nc.dram_tensor("sinf", [128, SEQ], F16, kind="ExternalInput").ap()
    y_d = nc.dram_tensor("y", [SEQ, DM], F16, kind="ExternalOutput").ap()

    # DVE-input view of a DT_MM AP (f32r bits are fp32 bits)
    VF = (lambda ap: ap.bitcast(F32)) if DT_MM == F32R else (lambda ap: ap)

    with tile.TileContext(nc) as tc:
        with tc.tile_pool(name="persist", bufs=1) as pp, \
             tc.tile_pool(name="ropep", bufs=3) as ropep, \
             tc.tile_pool(name="small", bufs=4) as sp, \
             tc.tile_pool(name="etp", bufs=LOOKAHEAD + 2) as etp, \
             tc.tile_pool(name="ysp", bufs=2) as ysp, \
             tc.tile_pool(name="ps_st", bufs=2, space="PSUM") as ps_st, \
             tc.tile_pool(name="ps_ot", bufs=2, space="PSUM") as ps_ot, \
             tc.tile_pool(name="ps_pj", bufs=2, space="PSUM") as ps_pj:

            # ---- resident tensors -------------------------------------
            # qt/kt live in ONE tile so the RoPE rotate-half swap covers
            # both with a single strided DMA per 32-partition block.
            qkt = pp.tile([128, 4 * SEQ], DT_MM, tag="qkt")
            qt = qkt[:, 0:2 * SEQ]
            kt = qkt[:, 2 * SEQ:4 * SEQ]
            v_sb = pp.tile([128, 16 * (HPC * 128)], DT_MM, tag="v")
            ht = pp.tile([128, 2 * SEQ], DT_MM, tag="ht")
            wo_sb = pp.tile([128, 2 * DM], DT_MM, tag="wo")
            # xt: one tile PER CHUNK so proj(c) depends only on its own DMA
            xt_c = [pp.tile([128, NK * 512], DT_MM, tag=f"xt{c}",
                            name=f"xt{c}")
                    for c in range(NJ)]
            wq_sb = pp.tile([128, NK * DH], DT_MM, tag="wq")
            wk_sb = pp.tile([128, NK * DH], DT_MM, tag="wk")
            wv_sb = pp.tile([128, NK * DH], DT_MM, tag="wv")
            cs_all = pp.tile([128, SEQ], F16, tag="cs")
            sn_all = pp.tile([128, SEQ], F16, tag="sn")
            tri = pp.tile([128, 128], DT_MM, tag="tri")

            xtv = xt_d.rearrange("(k p) s -> p k s", p=128)
            wq_s3 = wq_sb.rearrange("p (k d) -> p k d", d=DH)
            wk_s3 = wk_sb.rearrange("p (k d) -> p k d", d=DH)
            wv_s3 = wv_sb.rearrange("p (k d) -> p k d", d=DH)
            wo_s3 = wo_sb.rearrange("p (k d) -> p k d", d=DM)

            def xc3(c):
                return xt_c[c].rearrange("p (k s) -> p k s", s=512)

            # ---- input DMAs ------------------------------------------
            # All 5 engines issue at startup (each DIRECT2D costs ~700ns of
            # sequencer time; PE/ACT are idle until their first data lands
            # anyway). Pieces are ~128KB with >=1KB per-partition lines
            # (~45GB/s per channel), ordered by need-time: per-engine queue
            # slot s issues at ~7.2+0.7s us, lands ~2.8us later.
            def _wq(ks):
                return (wq_s3[:, ks, :], wq_d[:, ks, :])

            def _wk(ks):
                return (wk_s3[:, ks, :], wk_d[:, ks, :])

            def _wv(ks):
                return (wv_s3[:, ks, :], wv_d[:, ks, :])

            def _xt0(k, half=None):
                if half is None:
                    return (xc3(0)[:, k:k + 1, :], xtv[:, k:k + 1, 0:512])
                cs_ = slice(256 * half, 256 * half + 256)
                return (xc3(0)[:, k:k + 1, cs_], xtv[:, k:k + 1, cs_])

            def _xt1(k):
                return (xc3(1)[:, k:k + 1, :], xtv[:, k:k + 1, 512:1024])

            def _cs(c, t=None):
                sl = slice(512 * c, 512 * (c + (2 if t else 1)))
                return (cs_all[:, sl], cos_d[:, sl])

            def _sn(c, t=None):
                sl = slice(512 * c, 512 * (c + (2 if t else 1)))
                return (sn_all[:, sl], sin_d[:, sl])

            P2 = lambda a, b: slice(a, b)
            # Only SP(sync)/ACT(scalar) have HWDGE; gpsimd has SWDGE.
            # One engine's consecutive dma_starts land on different queues
            # and transfer CONCURRENTLY; the ~0.7us per-issue sequencer cost
            # is the startup constraint, so order by need-time. Scalar stops
            # after slot 7 (it must be free for the bootstrap proj drains).
            waves = {
                nc.sync: [_wq(P2(0, 1)), _wq(P2(1, 2)), _wq(P2(2, 4)),
                          _wq(P2(4, 6)), _wq(P2(6, 8)), _wk(P2(0, 2)),
                          _wk(P2(6, 8)), _sn(0), _wv(P2(6, 8)), _xt1(1),
                          _xt1(3), _xt1(5), _xt1(7), _sn(1), _sn(2, 't')],
                nc.scalar: [_xt0(0, 0), _xt0(2), _xt0(4), _xt0(6),
                            _wk(P2(2, 4)), _wv(P2(0, 2)), _cs(0)],
                nc.gpsimd: [_xt0(0, 1), _xt0(1), _xt0(3), _xt0(5),
                            _xt0(7), _wk(P2(4, 6)), _wv(P2(2, 4)),
                            _wv(P2(4, 6)), _xt1(0), _xt1(2), _xt1(4),
                            _xt1(6), _cs(1), _cs(2, 't')],
            }
            for si in range(15):
                for eng, lst in waves.items():
                    if si < len(lst):
                        o, i_ = lst[si]
                        eng.dma_start(out=o, in_=i_)

            h0, h1 = slice(0, 4), slice(4, 8)

            def late_loads(c):
                # late input loads (sync stays clear for rope swaps)
                if c == 0:
                    nc.gpsimd.dma_start(out=wo_s3[:, 0:1, :],
                                        in_=wo_d[:, 0:1, :])
                    nc.gpsimd.dma_start(out=wo_s3[:, 1:2, :],
                                        in_=wo_d[:, 1:2, :])
                    nc.gpsimd.dma_start(out=xc3(2)[:, h0, :],
                                        in_=xtv[:, h0, 1024:1536])
                    nc.gpsimd.dma_start(out=xc3(2)[:, h1, :],
                                        in_=xtv[:, h1, 1024:1536])
                elif c == 1:
                    nc.gpsimd.dma_start(out=xc3(3)[:, h0, :],
                                        in_=xtv[:, h0, 1536:2048])
                    nc.gpsimd.dma_start(out=xc3(3)[:, h1, :],
                                        in_=xtv[:, h1, 1536:2048])

            # ones-columns of v_sb are constant: write them ONCE here
            # (cols 0:64 of each of the 64 head-slots)
            v4 = v_sb.rearrange("p (s d) -> p s d", d=128)
            nc.gpsimd.memset(v4[:, :, 0:64], 1.0)
            # lower-triangular keep-mask for the causal diag blocks:
            # tri[p, c] = 1 where c >= p else 0 (multiplied into et on DVE)
            nc.gpsimd.memset(tri[:], 1.0)
            nc.gpsimd.affine_select(
                out=tri[:], in_=tri[:], compare_op=ALU.is_ge, fill=0.0,
                base=0, pattern=[[1, 128]], channel_multiplier=-1)

            def emit_proj_group(c, g, cp=None):
                """One projection group of chunk c.

                g 0..3: QK groups (chunk 0 orders them q,q,k,k so the wk
                DMAs get extra slack in the DMA-paced bootstrap); g 4..7:
                V seq-tile groups. cp: drain engine (ACT during bootstrap
                and when folded into an ACT-light attention stream, DVE
                otherwise).
                """
                xc = xt_c[c]
                if cp is None:
                    cp = nc.scalar if c <= 1 else nc.vector

                def drain(dst, src):
                    if cp is nc.scalar:
                        cp.copy(dst, src)
                    else:
                        cp.tensor_copy(dst, src)

                if g < 4:
                    if c == 0:
                        m, dst, w_sb = [(0, qt, wq_sb), (1, qt, wq_sb),
                                        (0, kt, wk_sb), (1, kt, wk_sb)][g]
                    else:
                        m, dst, w_sb = [(0, qt, wq_sb), (0, kt, wk_sb),
                                        (1, qt, wq_sb), (1, kt, wk_sb)][g]
                    ps = ps_pj.tile([128, 512], F32, tag="pj")
                    for k in range(NK):
                        nc.tensor.matmul(
                            ps[:],
                            w_sb[:, k * DH + m * 128: k * DH + (m + 1) * 128],
                            xc[:, k * 512:(k + 1) * 512],
                            start=(k == 0), stop=(k == NK - 1))
                    drain(dst[:, m * SEQ + c * 512: m * SEQ + (c + 1) * 512],
                          ps[:])
                else:
                    # V for seq tile 4c+t4, [sq 128, 4 head slots x 128].
                    # All slots hold [ones|dims]: PV outputs then carry the
                    # denominator on partitions 0:63 (where the DVE approx
                    # reciprocal is legal) and dims on 64:127. The ones
                    # cols were written once in the prologue.
                    t4 = g - 4
                    t = 4 * c + t4
                    ps = ps_pj.tile([128, 512], F32, tag="pj")
                    for k in range(NK):
                        nc.tensor.matmul(
                            ps[:, 0:DH],
                            xc[:, k * 512 + t4 * 128: k * 512 + t4 * 128 + 128],
                            wv_sb[:, k * DH:(k + 1) * DH],
                            start=(k == 0), stop=(k == NK - 1))
                    vt = v_sb[:, t * 512:(t + 1) * 512].rearrange(
                        "p (h d) -> p h d", d=128)
                    pv4 = ps[:, 0:DH].rearrange("p (h d) -> p h d", d=64)
                    drain(vt[:, :, 64:128], pv4[:])

            def emit_proj(c, cp=None):
                for g in range(8):
                    emit_proj_group(c, g, cp=cp)

            def emit_rope(c, ms=(0, 1)):
                # RoPE on QT/KT chunk c, in place. Head-dim pairs are
                # pre-permuted to [evens|odds] 32-row blocks (host-side
                # weight permute), so rotate-half = two 32-partition block
                # swaps done as 4 small SBUF DMAs per segment (per-block
                # DMAs keep the dep intervals precise — a fused strided DMA
                # false-serializes against later proj copies). Segment
                # order (q m0, k m0, q m1, k m1) lets attention's hp=0
                # start before the m=1 segments finish.
                cseg = slice(c * 512, (c + 1) * 512)
                segs = [s for s in ((0, qt), (2, kt), (1, qt), (3, kt))
                        if (s[0] % 2) in ms]
                for a, src in segs:
                    m = a % 2
                    base = m * SEQ + c * 512
                    seg = slice(base, base + 512)
                    t1 = ropep.tile([128, 512], F16, tag="t1")
                    nc.vector.tensor_mul(t1[:], VF(src[:, seg]),
                                         cs_all[:, cseg])
                    sw = ropep.tile([128, 512], DT_MM, tag="sw")
                    for blk in range(4):
                        sb_ = blk ^ 1
                        eng = nc.sync if blk % 2 == 0 else nc.gpsimd
                        eng.dma_start(
                            out=sw[blk * 32:(blk + 1) * 32, :],
                            in_=src[sb_ * 32:(sb_ + 1) * 32, seg])
                    sw2 = ropep.tile([128, 512], F16, tag="sw2")
                    nc.vector.tensor_mul(sw2[:], VF(sw[:]),
                                         sn_all[:, cseg])
                    nc.vector.tensor_add(src[:, seg], t1[:], sw2[:])

            def emit_oproj(j, last=False, t4s=range(4), act_drain=False):
                # Y[sq,dm] = H @ wo: lhsT = ht columns (weight reuse x2)
                for t4 in t4s:
                    ps0 = ps_pj.tile([128, 512], F32, tag="pj")
                    ps1 = ps_pj.tile([128, 512], F32, tag="pj")
                    for kk in range(2):
                        for n, psn in ((0, ps0), (1, ps1)):
                            nc.tensor.matmul(
                                psn[:],
                                ht[:, kk * SEQ + j * 512 + t4 * 128:
                                   kk * SEQ + j * 512 + (t4 + 1) * 128],
                                wo_sb[:, kk * DM + n * 512:
                                      kk * DM + (n + 1) * 512],
                                start=(kk == 0), stop=(kk == 1))
                    ys = ysp.tile([128, 1024], F16, tag="ys")
                    row = j * 512 + t4 * 128
                    if last:
                        # tail: drains split DVE+ACT (ACT idle after the
                        # last exp), y written by COLUMN halves so each
                        # DMA departs as soon as its own copy lands
                        nc.vector.tensor_copy(ys[:, 0:512], ps0[:])
                        nc.scalar.copy(ys[:, 512:1024], ps1[:])
                        # gpsimd carries no DMAs near the end of the kernel:
                        # its expensive SWDGE drain then runs concurrently
                        # with the attention tail instead of after it
                        ring = (nc.sync, nc.sync, nc.sync, nc.sync,
                                nc.sync, nc.scalar, nc.sync, nc.scalar)
                        ring[2 * t4].dma_start(
                            out=y_d[row:row + 128, 0:512], in_=ys[:, 0:512])
                        ring[2 * t4 + 1].dma_start(
                            out=y_d[row:row + 128, 512:1024],
                            in_=ys[:, 512:1024])
                    else:
                        nc.vector.tensor_copy(ys[:, 0:512], ps0[:])
                        if act_drain:
                            nc.scalar.copy(ys[:, 512:1024], ps1[:])
                        else:
                            nc.vector.tensor_copy(ys[:, 512:1024], ps1[:])
                        eng = nc.gpsimd if j == 0 else nc.sync
                        eng.dma_start(out=y_d[row:row + 128, :], in_=ys[:])

            def emit_attn(js, fillers=None, mid=None, tail=False):
                # js: sq-chunk indices processed back-to-back per hp half
                # (merging small chunks keeps the ACT exp pipeline dense).
                # fillers: PE work (proj groups / o_proj bursts) consumed
                # one per odd tile index — the ACT exp lookahead hides the
                # matmul detours. mid: called between the hp halves (used
                # to slot the next chunk's rope segments into the DVE queue
                # where they can't head-of-line-block the masks). tail:
                # final chunk — fuse the hp1 normalize with o_proj per
                # 128-col block to shorten the serial tail.
                fillers = list(fillers or [])
                secs = [(hp, j) for hp in range(2) for j in js]
                tiles = [(s, i) for s in range(len(secs))
                         for i in range(4 * (secs[s][1] + 1))]
                ots = {}
                ets = {}

                def st_exp(t):
                    s, i = tiles[t]
                    hp, j = secs[s]
                    jb = hp * SEQ + j * 512
                    r = i - 4 * j          # >= 0 on diagonal tiles
                    c0 = 128 * r if r >= 0 else 0
                    ib = hp * SEQ + i * 128
                    st = ps_st.tile([128, 1024], F32, tag="st")
                    nc.tensor.matmul(st[:, c0:512],
                                     kt[0:64, ib:ib + 128],
                                     qt[0:64, jb + c0:jb + 512],
                                     start=True, stop=True)
                    nc.tensor.matmul(st[:, 512 + c0:1024],
                                     kt[64:128, ib:ib + 128],
                                     qt[64:128, jb + c0:jb + 512],
                                     start=True, stop=True)
                    et = etp.tile([128, 1024], DT_MM, tag="et")
                    if r < 0 or c0 == 0:
                        # one fused 1024-col exp
                        nc.scalar.activation(et[:], st[:],
                                             AF.Exp, scale=0.125)
                    else:
                        # diag: single exp over both narrowed halves
                        # via a 3D AP (one ACT instr, not two)
                        st3 = st.rearrange("p (b q) -> p b q", q=512)
                        et3 = et.rearrange("p (b q) -> p b q", q=512)
                        nc.scalar.activation(et3[:, :, c0:512],
                                             st3[:, :, c0:512],
                                             AF.Exp, scale=0.125)
                    if r >= 0:
                        # zero above-diagonal inside the [128,128] diag
                        # block: multiply by the tri keep-mask on DVE
                        # (fp16 2x mode, ~170ns; keeps GPSIMD clear)
                        for b0 in (c0, 512 + c0):
                            nc.vector.tensor_mul(
                                et[:, b0:b0 + 128],
                                et[:, b0:b0 + 128], tri[:])
                    ets[t] = (et, c0)

                def pv(t):
                    s, i = tiles[t]
                    hp, j = secs[s]
                    nlive = 4 * (j + 1)
                    if i == 0:
                        ots[s] = (ps_ot.tile([128, 512], F32, tag="ot",
                                             name=f"otA{s}"),
                                  ps_ot.tile([128, 512], F32, tag="ot",
                                             name=f"otB{s}"))
                    otA, otB = ots[s]
                    et, c0 = ets.pop(t)
                    vb = i * (HPC * 128) + 2 * hp * 128
                    nc.tensor.matmul(otA[:, c0:512],
                                     v_sb[:, vb:vb + 128],
                                     et[:, c0:512],
                                     start=(i == 0), stop=(i == nlive - 1))
                    nc.tensor.matmul(otB[:, c0:512],
                                     v_sb[:, vb + 128:vb + 256],
                                     et[:, 512 + c0:1024],
                                     start=(i == 0), stop=(i == nlive - 1))

                def finish(s):
                    # normalize: denom rides rows 0:63, dims rows 64:127.
                    # rcp is base-0-aligned; the mul reads dims at base 64
                    # (both constructs HW-verified).
                    hp, j = secs[s]
                    jb = hp * SEQ + j * 512
                    otA, otB = ots.pop(s)
                    if tail and s == len(secs) - 1:
                        # last chunk: normalize per 128-col block, each
                        # immediately feeding its o_proj t4 burst — the
                        # serial normalize->oproj tail pipelines instead
                        for t4 in range(4):
                            csl = slice(t4 * 128, (t4 + 1) * 128)
                            hsl = slice(jb + t4 * 128, jb + (t4 + 1) * 128)
                            for ot, rows in ((otA, slice(0, 64)),
                                             (otB, slice(64, 128))):
                                rcp = sp.tile([64, 128], F32, tag="rcpt")
                                nc.vector.reciprocal_approx_fast(
                                    rcp[:], ot[0:64, csl])
                                nc.vector.tensor_mul(ht[rows, hsl],
                                                     ot[64:128, csl], rcp[:])
                            emit_oproj(j, last=True, t4s=(t4,))
                    else:
                        for ot, rows in ((otA, slice(0, 64)),
                                         (otB, slice(64, 128))):
                            rcp = sp.tile([64, 512], F32, tag="rcp")
                            nc.vector.reciprocal_approx_fast(rcp[:],
                                                             ot[0:64, :])
                            nc.vector.tensor_mul(ht[rows, jb:jb + 512],
                                                 ot[64:128, :], rcp[:])

                # flat tile stream: the exp lookahead spans section (hp/j)
                # boundaries, so the ACT pipeline never refills from empty
                # at an hp transition
                ntiles = len(tiles)
                stride = 2
                if fillers:
                    stride = max(2, (ntiles // len(fillers)) & ~1)
                for t in range(min(LOOKAHEAD, ntiles)):
                    st_exp(t)
                for t in range(ntiles):
                    if t + LOOKAHEAD < ntiles:
                        st_exp(t + LOOKAHEAD)
                    pv(t)
                    s, i = tiles[t]
                    # filler PE work rides inside this stream in small
                    # bursts (spread over the whole phase) so the ACT exp
                    # lookahead never drains during the matmul detours
                    if fillers and t % stride == 1:
                        fillers.pop(0)()
                    if i == 4 * (secs[s][1] + 1) - 1:
                        finish(s)
                        if mid is not None and s == len(secs) // 2 - 1:
                            mid()

            # ---- chunk-pipelined main loop ----------------------------
            # attn(0) is tiny (all-diagonal, mask/exp-paced) — merged into
            # attn(1)'s stream. proj(3) + the o_proj bursts ride as fillers
            # inside the attention streams (the only standalone PE phases
            # left are proj(0..2)); rope segments are emitted at the hp
            # boundaries so their swap-muls never head-of-line-block the
            # next masks on the DVE queue.
            emit_proj(0)
            emit_rope(0)
            late_loads(0)
            emit_proj(1)
            emit_rope(1)
            late_loads(1)
            # proj(2) drains on ACT (idle until attn01's first exps): on
            # the DVE they head-of-line-block behind rope(1)'s swap-muls
            emit_proj(2, cp=nc.scalar)
            fill01 = [lambda g=g: emit_proj_group(3, g, cp=nc.scalar)
                      for g in range(8)]
            fill01 += [lambda t=t: emit_oproj(0, t4s=(t,), act_drain=True)
                       for t in range(4)]
            emit_attn([0, 1], fillers=fill01,
                      mid=lambda: emit_rope(2, ms=(0,)))
            emit_rope(2, ms=(1,))
            emit_attn([2],
                      fillers=[lambda t=t: emit_oproj(1, t4s=(t,))
                               for t in range(4)],
                      mid=lambda: emit_rope(3, ms=(0,)))
            emit_rope(3, ms=(1,))
            emit_attn([3],
                      fillers=[lambda t=t: emit_oproj(2, t4s=(t,))
                               for t in range(4)],
                      tail=True)

    nc.compile()
    return nc


def _prep_inputs(x, Wq, Wk, Wv, Wo, token_positions):
    x = np.asarray(x, dtype=np.float32)
    Wq = np.asarray(Wq, dtype=np.float32)
    Wk = np.asarray(Wk, dtype=np.float32)
    Wv = np.asarray(Wv, dtype=np.float32)
    Wo = np.asarray(Wo, dtype=np.float32)
    pos = np.asarray(token_positions).astype(np.float32)

    inv = 1.0 / (ROPE_THETA ** (np.arange(0, DK, 2, dtype=np.float32) / DK))
    freqs = pos[:, None] * inv[None, :]              # [SEQ, 32]
    cos_t, sin_t = np.cos(freqs).T, np.sin(freqs).T  # [32, SEQ]
    cosf = np.ascontiguousarray(np.tile(cos_t, (4, 1)), dtype=np.float16)
    sinf = np.tile(sin_t, (4, 1)).astype(np.float32)
    sinf[0:32] *= -1.0   # evens block gets -sin; odds +sin
    sinf[64:96] *= -1.0
    sinf = np.ascontiguousarray(sinf.astype(np.float16))

    def _pkd(wT):
        # [K*128, D] (contraction-major) -> [128, K, D] partition-major so
        # each k-pair DMA piece is >=1KB-contiguous per partition
        k = wT.shape[0] // 128
        return np.ascontiguousarray(
            _round_mm(wT).reshape(k, 128, -1).transpose(1, 0, 2))

    perm = np.concatenate([np.arange(0, 64, 2), np.arange(1, 64, 2)])
    in_maps = []
    for c in range(NCORES):
        b, g = divmod(c, GROUPS)
        rows = slice(g * DH, (g + 1) * DH)
        wq_s = Wq[rows, :].reshape(HPC, DK, DM)[:, perm, :].reshape(DH, DM)
        wk_s = Wk[rows, :].reshape(HPC, DK, DM)[:, perm, :].reshape(DH, DM)
        in_maps.append({
            "xt": _round_mm(x[b].T),
            "wq": _pkd(wq_s.T),
            "wk": _pkd(wk_s.T),
            "wv": _pkd(Wv[rows, :].T),
            "wo": _pkd(Wo[:, rows].T),
            "cosf": cosf,
            "sinf": sinf,
        })
    return in_maps


def kernel(x, Wq, Wk, Wv, Wo, token_positions):
    global _NC, LAST_RESULTS
    if _NC is None:
        _NC = _build()
    in_maps = _prep_inputs(x, Wq, Wk, Wv, Wo, token_positions)
    res = run_bass_kernel_spmd(_NC, in_maps, list(range(NCORES)), trace=TRACE)
    LAST_RESULTS = res
    y = np.empty((BATCH, SEQ, DM), dtype=np.float32)
    for b in range(BATCH):
        acc = res.results[4 * b]["y"].astype(np.float32)
        for g in range(1, GROUPS):
            acc += res.results[4 * b + g]["y"].astype(np.float32)
        y[b] = acc
    return y

